# revision 11
# baseline (speedup 1.0000x reference)
"""Trainium2 Bass kernel for nn_EmbeddingHead (avgpool -> linear head -> softmax stats).

Reference computation (B=256, C=2048, H=16, W=8, NUM_CLASSES=10000):
    neck   = features.mean(axis=(2,3))                  # [B, C]
    logits = neck @ weight.T                            # [B, NUM_CLASSES]
    soft   = softmax(logits); right_prob = soft[b, targets[b]]
    new_weight = t * tanh(1.2 * right_prob / 0) / sum(...)   # == ones (tanh(inf)=1)
    returns (logits, logits * 1.0, neck, new_weight[None, :])

Sharding (8 NeuronCores):
  - features: data-parallel over batch (32 rows per core). Each core pools its
    own rows; pooling is split across VectorE (segmented reduce) and ScalarE
    (activation accumulate) so it keeps pace with the DMA stream.
  - weight: tensor-parallel over classes (1250 per core), host-pretransposed to
    [C, cls_shard] (and pre-divided by H*W so the device pools a plain sum).
  - pooled neck shards are AllGather'd in phases over channel groups so the
    FP32 matmul overlaps the feature stream; each core computes logits[:, its
    1250 classes] accumulating over 16 K-tiles in PSUM.
  - Channel order within a K-tile is a "comb" (c = base + p*n_g + j) so each
    feature DMA reads n_g*512B contiguous per partition (large DMA packets);
    the weight is sliced with the same comb so the contraction still matches.
  - host concatenates logits shards, rebuilds neck (undoing the comb
    permutation), and computes the trivial softmax/new_weight tail in numpy.
"""

import os
import sys

import numpy as np

sys.path.insert(0, "/opt/trn_rl_repo")

B, C, H, W = 256, 2048, 16, 8
HW = H * W
NUM_CLASSES = 10000
N_CORES = 8
B_LOC = B // N_CORES          # 32 batch rows per core
CLS_LOC = NUM_CLASSES // N_CORES  # 1250 classes per core
P = 128
N_CT = C // P                 # 16 K-tiles of 128 channels

# Channel-group sizes (in units of 128-channel K-tiles) for the phased
# AllGather. Must sum to N_CT. Early groups big (long contiguous DMA slabs),
# later groups small (short tail after the last AG).
GROUPS = [8, 5, 3]

# batch rows pooled on ScalarE (rest on VectorE) - balances the two engines.
# ScalarE gets the EARLY rows so its slower per-op cost is hidden mid-stream
# and the group's pool completion is gated by VectorE only.
ACT_B = 9

# N-chunks of the per-core class shard (PSUM bank = 512 fp32)
N_CHUNKS = [(0, 512), (512, 512), (1024, 226)]

_CACHE = {}


def _group_bases():
    bases = []
    t0 = 0
    for n in GROUPS:
        bases.append(t0)
        t0 += n
    assert t0 == N_CT
    return bases


def _build():
    """Build + compile the SPMD Bass graph once."""
    if "nc" in _CACHE:
        return _CACHE["nc"]

    import concourse.bass as bass
    import concourse.mybir as mybir
    import concourse.tile as tile
    from concourse import bacc

    f32 = mybir.dt.float32

    nc = bacc.Bacc(
        "TRN2",
        target_bir_lowering=False,
        debug=False,
        num_devices=N_CORES,
    )

    feat_in = nc.dram_tensor("features", [B_LOC, C, HW], f32, kind="ExternalInput")
    wt_in = nc.dram_tensor("weight_t", [C, CLS_LOC], f32, kind="ExternalInput")
    logits_out = nc.dram_tensor("logits", [2, P, CLS_LOC], f32, kind="ExternalOutput")
    neck_out = nc.dram_tensor("neck_out", [P, N_CT * B_LOC], f32, kind="ExternalOutput")

    rg = [list(range(N_CORES))]
    bases = _group_bases()

    with tile.TileContext(nc) as tc:
        with (
            tc.tile_pool(name="wpool", bufs=1) as wpool,
            tc.tile_pool(name="fpool", bufs=10) as fpool,
            tc.tile_pool(name="fpool_act", bufs=9) as fpool_act,
            tc.tile_pool(name="spool", bufs=4) as spool,
            tc.tile_pool(name="npool", bufs=1) as npool,
            tc.tile_pool(name="ltpool", bufs=1) as ltpool,
            tc.tile_pool(name="opool", bufs=2) as opool,
            tc.tile_pool(name="psumpool", bufs=1, space="PSUM") as psumpool,
            tc.tile_pool(name="drampool", bufs=1, space="DRAM") as drampool,
        ):
            # persistent tiles
            bf16 = mybir.dt.bfloat16
            w_sb = wpool.tile([P, N_CT, CLS_LOC], bf16, name="w_sb")
            neck_local = npool.tile([P, N_CT * B_LOC], f32, name="neck_local")
            nl3 = neck_local[:].rearrange("p (c b) -> p c b", b=B_LOC)

            lhsT = [ltpool.tile([P, B], bf16, name=f"lhsT{t}") for t in range(N_CT)]
            psum_tiles = {}
            for m in range(2):
                for nci, (noff, nsz) in enumerate(N_CHUNKS):
                    psum_tiles[(m, nci)] = psumpool.tile(
                        [P, nsz], f32, name=f"ps_{m}_{nci}"
                    )

            for g, cnt in enumerate(GROUPS):
                t0 = bases[g]
                c_lo = t0 * P          # first channel row of this group
                n_rows = cnt * P       # channel rows in this group

                # weight chunk for this group, comb layout:
                # w_sb[p, t0+j, n] = wT[c_lo + p*cnt + j, n]
                w_src = wt_in.ap()[c_lo : c_lo + n_rows, :].rearrange(
                    "(p j) n -> p j n", j=cnt
                )
                nc.gpsimd.dma_start(w_sb[:, t0 : t0 + cnt, :], w_src)

                # feature DMAs (slab-contiguous: cnt*512B per partition) and
                # pooling. Partition p holds channels c_lo + p*cnt + j.
                for b in range(B_LOC):
                    if b >= ACT_B:
                        fb = fpool.tile([P, cnt, HW], f32, name="fb", tag="fb")
                    else:
                        fb = fpool_act.tile(
                            [P, cnt, HW], f32, name="fba", tag="fba"
                        )
                    src = feat_in.ap()[b, c_lo : c_lo + n_rows, :].rearrange(
                        "(p j) h -> p j h", p=P
                    )
                    nc.sync.dma_start(fb[:], src)
                    if b >= ACT_B:
                        nc.vector.tensor_reduce(
                            out=nl3[:, t0 : t0 + cnt, b],
                            in_=fb[:],
                            axis=mybir.AxisListType.X,
                            op=mybir.AluOpType.add,
                        )
                    else:
                        scratch = spool.tile(
                            [P, HW], f32, name="scr", tag="scr"
                        )
                        for j in range(cnt):
                            nc.scalar.activation(
                                out=scratch[:],
                                in_=fb[:, j, :],
                                func=mybir.ActivationFunctionType.Copy,
                                accum_out=nl3[:, t0 + j, b : b + 1],
                            )

                # AllGather this group's pooled neck slice
                ag_in = drampool.tile([P, cnt * B_LOC], f32, name=f"ag_in{g}")
                ag_out = drampool.tile(
                    [N_CORES * P, cnt * B_LOC], f32, name=f"ag_out{g}",
                    addr_space="Shared",
                )
                nc.scalar.dma_start(
                    ag_in[:], neck_local[:, t0 * B_LOC : (t0 + cnt) * B_LOC]
                )
                nc.gpsimd.collective_compute(
                    "AllGather",
                    mybir.AluOpType.bypass,
                    ins=[ag_in[:].opt()],
                    outs=[ag_out[:].opt()],
                    replica_groups=rg,
                )
                # read back gathered neck as lhsT tiles [K=128, M=256]
                ag4 = ag_out[:].rearrange("(r p) (c b) -> p c r b", p=P, b=B_LOC)
                for j in range(cnt):
                    t = t0 + j
                    dst = lhsT[t][:].rearrange("p (r b) -> p r b", b=B_LOC)
                    nc.gpsimd.dma_start(dst, ag4[:, j, :, :])

                # matmuls for this group's K-tiles (accumulate into psum)
                for j in range(cnt):
                    t = t0 + j
                    for m in range(2):
                        for nci, (noff, nsz) in enumerate(N_CHUNKS):
                            nc.tensor.matmul(
                                psum_tiles[(m, nci)][:],
                                lhsT=lhsT[t][:, m * P : (m + 1) * P],
                                rhs=w_sb[:, t, noff : noff + nsz],
                                start=(t == 0),
                                stop=(t == N_CT - 1),
                            )

            # epilogue: PSUM -> SBUF -> DRAM
            for m in range(2):
                for nci, (noff, nsz) in enumerate(N_CHUNKS):
                    osb = opool.tile([P, nsz], f32, name="osb", tag="osb")
                    nc.scalar.copy(out=osb[:], in_=psum_tiles[(m, nci)][:])
                    nc.sync.dma_start(
                        logits_out.ap()[m, :, noff : noff + nsz], osb[:]
                    )
            nc.sync.dma_start(neck_out.ap()[:, :], neck_local[:])

    nc.compile()
    _CACHE["nc"] = nc
    return nc


def _channel_of_tile():
    """c[p, t] = global channel held at partition p of K-tile t."""
    bases = _group_bases()
    cmap = np.zeros((P, N_CT), dtype=np.int64)
    for g, cnt in enumerate(GROUPS):
        t0 = bases[g]
        for j in range(cnt):
            cmap[:, t0 + j] = t0 * P + np.arange(P) * cnt + j
    return cmap


def _shard_inputs(features, weight):
    features = np.ascontiguousarray(features, dtype=np.float32).reshape(B, C, HW)
    weight = np.asarray(weight, dtype=np.float32)
    in_maps = []
    for i in range(N_CORES):
        f_i = np.ascontiguousarray(features[i * B_LOC : (i + 1) * B_LOC])
        w_i = np.ascontiguousarray(
            weight[i * CLS_LOC : (i + 1) * CLS_LOC, :].T
        ) / np.float32(HW)
        in_maps.append({"features": f_i, "weight_t": w_i})
    return in_maps


def _assemble(results):
    # logits: core i holds classes [i*1250, (i+1)*1250) for all 256 rows
    logits = np.concatenate(
        [results[i]["logits"].reshape(B, CLS_LOC) for i in range(N_CORES)], axis=1
    ).astype(np.float32, copy=False)
    # neck: core i holds pooled SUM for batch rows [i*32, (i+1)*32), layout
    # [p, t, b] with channel cmap[p, t]
    cmap = _channel_of_tile()  # [P, N_CT]
    neck_parts = []
    for i in range(N_CORES):
        arr = results[i]["neck_out"].reshape(P, N_CT, B_LOC)
        part = np.empty((B_LOC, C), dtype=np.float32)
        # part[b, cmap[p,t]] = arr[p, t, b]
        part[:, cmap.reshape(-1)] = arr.reshape(P * N_CT, B_LOC).T
        neck_parts.append(part)
    neck = np.concatenate(neck_parts, axis=0) / np.float32(HW)
    return logits, neck


def _softmax_tail(logits, targets):
    """Faithful replication of the reference's softmax/new_weight path."""
    t = logits.shape[0]
    mx = logits.max(axis=1, keepdims=True)
    e = np.exp(logits - mx)
    denom = e.sum(axis=1)
    tgt = np.asarray(targets).astype(np.int64).reshape(-1)
    right_prob = e[np.arange(t), tgt] / denom  # [t]
    mean_sl = right_prob.astype(np.float32)
    var_sl = np.zeros_like(mean_sl)
    with np.errstate(divide="ignore", invalid="ignore"):
        con = mean_sl / (var_sl * np.float32(1e4))
    ri = np.tanh(np.float32(1.2) * con).astype(np.float32)
    new_weight = (np.float32(t) * ri / ri.sum())[None, :].astype(np.float32)
    return new_weight


def kernel(features, targets, weight, _trace=False, _extra=None):
    from concourse.bass_utils import run_bass_kernel_spmd

    nc = _build()
    in_maps = _shard_inputs(features, weight)
    res = run_bass_kernel_spmd(
        nc, in_maps, core_ids=list(range(N_CORES)), trace=_trace
    )
    if _extra is not None:
        _extra["bass_results"] = res
    logits, neck = _assemble(res.results)
    new_weight = _softmax_tail(logits, targets)
    cls_outputs = logits
    pred_class_logits = logits * np.float32(1.0)
    return cls_outputs, pred_class_logits, neck, new_weight


# revision 12
# speedup vs baseline: 1.0142x; 1.0142x over previous
"""Trainium2 Bass kernel for nn_EmbeddingHead (avgpool -> linear head -> softmax stats).

Reference computation (B=256, C=2048, H=16, W=8, NUM_CLASSES=10000):
    neck   = features.mean(axis=(2,3))                  # [B, C]
    logits = neck @ weight.T                            # [B, NUM_CLASSES]
    soft   = softmax(logits); right_prob = soft[b, targets[b]]
    new_weight = t * tanh(1.2 * right_prob / 0) / sum(...)   # == ones (tanh(inf)=1)
    returns (logits, logits * 1.0, neck, new_weight[None, :])

Sharding (8 NeuronCores):
  - features: data-parallel over batch (32 rows per core). Each core pools its
    own rows; pooling is split across VectorE (segmented reduce) and ScalarE
    (activation accumulate) so it keeps pace with the DMA stream.
  - weight: tensor-parallel over classes (1250 per core), host-pretransposed to
    [C, cls_shard] (and pre-divided by H*W so the device pools a plain sum).
  - pooled neck shards are AllGather'd in phases over channel groups so the
    FP32 matmul overlaps the feature stream; each core computes logits[:, its
    1250 classes] accumulating over 16 K-tiles in PSUM.
  - Channel order within a K-tile is a "comb" (c = base + p*n_g + j) so each
    feature DMA reads n_g*512B contiguous per partition (large DMA packets);
    the weight is sliced with the same comb so the contraction still matches.
  - host concatenates logits shards, rebuilds neck (undoing the comb
    permutation), and computes the trivial softmax/new_weight tail in numpy.
"""

import os
import sys

import numpy as np

sys.path.insert(0, "/opt/trn_rl_repo")

B, C, H, W = 256, 2048, 16, 8
HW = H * W
NUM_CLASSES = 10000
N_CORES = 8
B_LOC = B // N_CORES          # 32 batch rows per core
CLS_LOC = NUM_CLASSES // N_CORES  # 1250 classes per core
P = 128
N_CT = C // P                 # 16 K-tiles of 128 channels

# Channel-group sizes (in units of 128-channel K-tiles) for the phased
# AllGather. Must sum to N_CT. Early groups big (long contiguous DMA slabs),
# later groups small (short tail after the last AG).
GROUPS = [8, 5, 3]

# batch rows pooled on ScalarE (rest on VectorE) - balances the two engines.
# ScalarE gets the EARLY rows so its slower per-op cost is hidden mid-stream
# and the group's pool completion is gated by VectorE only.
ACT_B = 9

# N-chunks of the per-core class shard (PSUM bank = 512 fp32)
N_CHUNKS = [(0, 512), (512, 512), (1024, 226)]

_CACHE = {}


def _group_bases():
    bases = []
    t0 = 0
    for n in GROUPS:
        bases.append(t0)
        t0 += n
    assert t0 == N_CT
    return bases


def _build():
    """Build + compile the SPMD Bass graph once."""
    if "nc" in _CACHE:
        return _CACHE["nc"]

    import concourse.bass as bass
    import concourse.mybir as mybir
    import concourse.tile as tile
    from concourse import bacc

    f32 = mybir.dt.float32

    nc = bacc.Bacc(
        "TRN2",
        target_bir_lowering=False,
        debug=False,
        num_devices=N_CORES,
    )

    feat_in = nc.dram_tensor("features", [B_LOC, C, HW], f32, kind="ExternalInput")
    wt_in = nc.dram_tensor("weight_t", [C, CLS_LOC], f32, kind="ExternalInput")
    logits_out = nc.dram_tensor("logits", [2, P, CLS_LOC], f32, kind="ExternalOutput")
    neck_out = nc.dram_tensor("neck_out", [P, N_CT * B_LOC], f32, kind="ExternalOutput")

    rg = [list(range(N_CORES))]
    bases = _group_bases()

    with tile.TileContext(nc) as tc:
        with (
            tc.tile_pool(name="wpool", bufs=1) as wpool,
            tc.tile_pool(name="fpool", bufs=10) as fpool,
            tc.tile_pool(name="fpool_act", bufs=9) as fpool_act,
            tc.tile_pool(name="spool", bufs=4) as spool,
            tc.tile_pool(name="npool", bufs=1) as npool,
            tc.tile_pool(name="ltpool", bufs=1) as ltpool,
            tc.tile_pool(name="opool", bufs=2) as opool,
            tc.tile_pool(name="psumpool", bufs=1, space="PSUM") as psumpool,
            tc.tile_pool(name="drampool", bufs=1, space="DRAM") as drampool,
        ):
            # persistent tiles
            bf16 = mybir.dt.bfloat16
            w_sb = wpool.tile([P, N_CT, CLS_LOC], bf16, name="w_sb")
            neck_local = npool.tile([P, N_CT * B_LOC], f32, name="neck_local")
            nl3 = neck_local[:].rearrange("p (c b) -> p c b", b=B_LOC)

            # one gathered-neck staging tile per group: [p, c_local, r*32+b]
            lhsT_g = [
                ltpool.tile([P, cnt, B], bf16, name=f"lhsTg{g}")
                for g, cnt in enumerate(GROUPS)
            ]
            psum_tiles = {}
            for m in range(2):
                for nci, (noff, nsz) in enumerate(N_CHUNKS):
                    psum_tiles[(m, nci)] = psumpool.tile(
                        [P, nsz], f32, name=f"ps_{m}_{nci}"
                    )

            for g, cnt in enumerate(GROUPS):
                t0 = bases[g]
                c_lo = t0 * P          # first channel row of this group
                n_rows = cnt * P       # channel rows in this group

                # weight chunk for this group, comb layout:
                # w_sb[p, t0+j, n] = wT[c_lo + p*cnt + j, n]
                w_src = wt_in.ap()[c_lo : c_lo + n_rows, :].rearrange(
                    "(p j) n -> p j n", j=cnt
                )
                nc.gpsimd.dma_start(w_sb[:, t0 : t0 + cnt, :], w_src)

                # feature DMAs (slab-contiguous: cnt*512B per partition) and
                # pooling. Partition p holds channels c_lo + p*cnt + j.
                for b in range(B_LOC):
                    if b >= ACT_B:
                        fb = fpool.tile([P, cnt, HW], f32, name="fb", tag="fb")
                    else:
                        fb = fpool_act.tile(
                            [P, cnt, HW], f32, name="fba", tag="fba"
                        )
                    src = feat_in.ap()[b, c_lo : c_lo + n_rows, :].rearrange(
                        "(p j) h -> p j h", p=P
                    )
                    nc.sync.dma_start(fb[:], src)
                    if b >= ACT_B:
                        nc.vector.tensor_reduce(
                            out=nl3[:, t0 : t0 + cnt, b],
                            in_=fb[:],
                            axis=mybir.AxisListType.X,
                            op=mybir.AluOpType.add,
                        )
                    else:
                        scratch = spool.tile(
                            [P, HW], f32, name="scr", tag="scr"
                        )
                        for j in range(cnt):
                            nc.scalar.activation(
                                out=scratch[:],
                                in_=fb[:, j, :],
                                func=mybir.ActivationFunctionType.Copy,
                                accum_out=nl3[:, t0 + j, b : b + 1],
                            )

                # AllGather this group's pooled neck slice
                ag_in = drampool.tile([P, cnt * B_LOC], f32, name=f"ag_in{g}")
                ag_out = drampool.tile(
                    [N_CORES * P, cnt * B_LOC], f32, name=f"ag_out{g}",
                    addr_space="Shared",
                )
                nc.scalar.dma_start(
                    ag_in[:], neck_local[:, t0 * B_LOC : (t0 + cnt) * B_LOC]
                )
                nc.gpsimd.collective_compute(
                    "AllGather",
                    mybir.AluOpType.bypass,
                    ins=[ag_in[:].opt()],
                    outs=[ag_out[:].opt()],
                    replica_groups=rg,
                )
                # read back the whole gathered group in ONE SWDGE DMA
                # (f32 -> bf16 cast): dst[p, c, r, b] <- ag_out[(r p), (c b)]
                ag4 = ag_out[:].rearrange("(r p) (c b) -> p c r b", p=P, b=B_LOC)
                dst4 = lhsT_g[g][:].rearrange("p c (r b) -> p c r b", b=B_LOC)
                nc.gpsimd.dma_start(dst4, ag4)

                # matmuls for this group's K-tiles (accumulate into psum)
                for j in range(cnt):
                    t = t0 + j
                    for m in range(2):
                        for nci, (noff, nsz) in enumerate(N_CHUNKS):
                            nc.tensor.matmul(
                                psum_tiles[(m, nci)][:],
                                lhsT=lhsT_g[g][:, j, m * P : (m + 1) * P],
                                rhs=w_sb[:, t, noff : noff + nsz],
                                start=(t == 0),
                                stop=(t == N_CT - 1),
                            )

            # epilogue: PSUM -> SBUF -> DRAM
            for m in range(2):
                for nci, (noff, nsz) in enumerate(N_CHUNKS):
                    osb = opool.tile([P, nsz], f32, name="osb", tag="osb")
                    nc.scalar.copy(out=osb[:], in_=psum_tiles[(m, nci)][:])
                    nc.sync.dma_start(
                        logits_out.ap()[m, :, noff : noff + nsz], osb[:]
                    )
            nc.sync.dma_start(neck_out.ap()[:, :], neck_local[:])

    nc.compile()
    _CACHE["nc"] = nc
    return nc


def _channel_of_tile():
    """c[p, t] = global channel held at partition p of K-tile t."""
    bases = _group_bases()
    cmap = np.zeros((P, N_CT), dtype=np.int64)
    for g, cnt in enumerate(GROUPS):
        t0 = bases[g]
        for j in range(cnt):
            cmap[:, t0 + j] = t0 * P + np.arange(P) * cnt + j
    return cmap


def _shard_inputs(features, weight):
    features = np.ascontiguousarray(features, dtype=np.float32).reshape(B, C, HW)
    weight = np.asarray(weight, dtype=np.float32)
    in_maps = []
    for i in range(N_CORES):
        f_i = np.ascontiguousarray(features[i * B_LOC : (i + 1) * B_LOC])
        w_i = np.ascontiguousarray(
            weight[i * CLS_LOC : (i + 1) * CLS_LOC, :].T
        ) / np.float32(HW)
        in_maps.append({"features": f_i, "weight_t": w_i})
    return in_maps


def _assemble(results):
    # logits: core i holds classes [i*1250, (i+1)*1250) for all 256 rows
    logits = np.concatenate(
        [results[i]["logits"].reshape(B, CLS_LOC) for i in range(N_CORES)], axis=1
    ).astype(np.float32, copy=False)
    # neck: core i holds pooled SUM for batch rows [i*32, (i+1)*32), layout
    # [p, t, b] with channel cmap[p, t]
    cmap = _channel_of_tile()  # [P, N_CT]
    neck_parts = []
    for i in range(N_CORES):
        arr = results[i]["neck_out"].reshape(P, N_CT, B_LOC)
        part = np.empty((B_LOC, C), dtype=np.float32)
        # part[b, cmap[p,t]] = arr[p, t, b]
        part[:, cmap.reshape(-1)] = arr.reshape(P * N_CT, B_LOC).T
        neck_parts.append(part)
    neck = np.concatenate(neck_parts, axis=0) / np.float32(HW)
    return logits, neck


def _softmax_tail(logits, targets):
    """Faithful replication of the reference's softmax/new_weight path."""
    t = logits.shape[0]
    mx = logits.max(axis=1, keepdims=True)
    e = np.exp(logits - mx)
    denom = e.sum(axis=1)
    tgt = np.asarray(targets).astype(np.int64).reshape(-1)
    right_prob = e[np.arange(t), tgt] / denom  # [t]
    mean_sl = right_prob.astype(np.float32)
    var_sl = np.zeros_like(mean_sl)
    with np.errstate(divide="ignore", invalid="ignore"):
        con = mean_sl / (var_sl * np.float32(1e4))
    ri = np.tanh(np.float32(1.2) * con).astype(np.float32)
    new_weight = (np.float32(t) * ri / ri.sum())[None, :].astype(np.float32)
    return new_weight


def kernel(features, targets, weight, _trace=False, _extra=None):
    from concourse.bass_utils import run_bass_kernel_spmd

    nc = _build()
    in_maps = _shard_inputs(features, weight)
    res = run_bass_kernel_spmd(
        nc, in_maps, core_ids=list(range(N_CORES)), trace=_trace
    )
    if _extra is not None:
        _extra["bass_results"] = res
    logits, neck = _assemble(res.results)
    new_weight = _softmax_tail(logits, targets)
    cls_outputs = logits
    pred_class_logits = logits * np.float32(1.0)
    return cls_outputs, pred_class_logits, neck, new_weight


# revision 15
# speedup vs baseline: 1.1037x; 1.0882x over previous
"""Trainium2 Bass kernel for nn_EmbeddingHead (avgpool -> linear head -> softmax stats).

Reference computation (B=256, C=2048, H=16, W=8, NUM_CLASSES=10000):
    neck   = features.mean(axis=(2,3))                  # [B, C]
    logits = neck @ weight.T                            # [B, NUM_CLASSES]
    soft   = softmax(logits); right_prob = soft[b, targets[b]]
    new_weight = t * tanh(1.2 * right_prob / 0) / sum(...)   # == ones (tanh(inf)=1)
    returns (logits, logits * 1.0, neck, new_weight[None, :])

Sharding (8 NeuronCores):
  - features: data-parallel over batch (32 rows per core). Each core pools its
    own rows; pooling is split across VectorE (segmented reduce) and ScalarE
    (activation accumulate) so it keeps pace with the DMA stream.
  - weight: tensor-parallel over classes (1250 per core), host-pretransposed to
    [C, cls_shard] (and pre-divided by H*W so the device pools a plain sum).
  - pooled neck shards are AllGather'd in phases over channel groups; each
    core computes logits[:, its 1250 classes] with bf16 matmuls (f32->bf16
    cast during the SWDGE weight/readback DMAs, fp32 PSUM accumulate) over
    16 K-tiles.
  - Channel order within a K-tile is a "comb" (c = base + p*n_g + j) so each
    feature DMA reads n_g*512B contiguous per partition (large DMA packets);
    the weight is sliced with the same comb so the contraction still matches.
  - host concatenates logits shards, rebuilds neck (undoing the comb
    permutation), and computes the trivial softmax/new_weight tail in numpy.
"""

import sys

import numpy as np

sys.path.insert(0, "/opt/trn_rl_repo")

B, C, H, W = 256, 2048, 16, 8
HW = H * W
NUM_CLASSES = 10000
N_CORES = 8
B_LOC = B // N_CORES          # 32 batch rows per core
CLS_LOC = NUM_CLASSES // N_CORES  # 1250 classes per core
P = 128
N_CT = C // P                 # 16 K-tiles of 128 channels

# Channel-group sizes (in units of 128-channel K-tiles) for the phased
# AllGather. Must sum to N_CT. Early groups big (long contiguous DMA slabs),
# later groups small (short tail after the last AG).
GROUPS = [8, 5, 3]

# batch rows pooled on ScalarE (rest on VectorE) - balances the two engines.
# ScalarE gets the EARLY rows so its slower per-op cost is hidden mid-stream
# and the group's pool completion is gated by VectorE only.
ACT_B = 9

# N-chunks of the per-core class shard (PSUM bank = 512 fp32)
N_CHUNKS = [(0, 512), (512, 512), (1024, 226)]

_CACHE = {}


def _group_bases():
    bases = []
    t0 = 0
    for n in GROUPS:
        bases.append(t0)
        t0 += n
    assert t0 == N_CT
    return bases


def _build():
    """Build + compile the SPMD Bass graph once."""
    if "nc" in _CACHE:
        return _CACHE["nc"]

    import concourse.bass as bass
    import concourse.mybir as mybir
    import concourse.tile as tile
    from concourse import bacc

    f32 = mybir.dt.float32

    nc = bacc.Bacc(
        "TRN2",
        target_bir_lowering=False,
        debug=False,
        num_devices=N_CORES,
    )

    feat_in = nc.dram_tensor("features", [B_LOC, C, HW], f32, kind="ExternalInput")
    wt_in = nc.dram_tensor("weight_t", [C, CLS_LOC], f32, kind="ExternalInput")
    logits_out = nc.dram_tensor("logits", [2, P, CLS_LOC], f32, kind="ExternalOutput")
    neck_out = nc.dram_tensor("neck_out", [P, N_CT * B_LOC], f32, kind="ExternalOutput")

    rg = [list(range(N_CORES))]
    bases = _group_bases()

    with tile.TileContext(nc) as tc:
        with (
            tc.tile_pool(name="wpool", bufs=1) as wpool,
            tc.tile_pool(name="fpool", bufs=10) as fpool,
            tc.tile_pool(name="fpool_act", bufs=9) as fpool_act,
            tc.tile_pool(name="spool", bufs=4) as spool,
            tc.tile_pool(name="npool", bufs=1) as npool,
            tc.tile_pool(name="ltpool", bufs=1) as ltpool,
            tc.tile_pool(name="opool", bufs=2) as opool,
            tc.tile_pool(name="psumpool", bufs=1, space="PSUM") as psumpool,
            tc.tile_pool(name="drampool", bufs=1, space="DRAM") as drampool,
        ):
            # persistent tiles
            bf16 = mybir.dt.bfloat16
            w_sb = wpool.tile([P, N_CT, CLS_LOC], bf16, name="w_sb")
            neck_local = npool.tile([P, N_CT * B_LOC], f32, name="neck_local")
            nl3 = neck_local[:].rearrange("p (c b) -> p c b", b=B_LOC)

            # one gathered-neck staging tile per group: [p, c_local, r*32+b]
            lhsT_g = [
                ltpool.tile([P, cnt, B], bf16, name=f"lhsTg{g}")
                for g, cnt in enumerate(GROUPS)
            ]
            psum_tiles = {}
            for m in range(2):
                for nci, (noff, nsz) in enumerate(N_CHUNKS):
                    psum_tiles[(m, nci)] = psumpool.tile(
                        [P, nsz], f32, name=f"ps_{m}_{nci}"
                    )

            for g, cnt in enumerate(GROUPS):
                t0 = bases[g]
                c_lo = t0 * P          # first channel row of this group
                n_rows = cnt * P       # channel rows in this group

                # weight chunk for this group, comb layout:
                # w_sb[p, t0+j, n] = wT[c_lo + p*cnt + j, n]
                w_src = wt_in.ap()[c_lo : c_lo + n_rows, :].rearrange(
                    "(p j) n -> p j n", j=cnt
                )
                nc.gpsimd.dma_start(w_sb[:, t0 : t0 + cnt, :], w_src)

                # feature DMAs (slab-contiguous: cnt*512B per partition) and
                # pooling. Partition p holds channels c_lo + p*cnt + j.
                for b in range(B_LOC):
                    if b >= ACT_B:
                        fb = fpool.tile([P, cnt, HW], f32, name="fb", tag="fb")
                    else:
                        fb = fpool_act.tile(
                            [P, cnt, HW], f32, name="fba", tag="fba"
                        )
                    src = feat_in.ap()[b, c_lo : c_lo + n_rows, :].rearrange(
                        "(p j) h -> p j h", p=P
                    )
                    nc.sync.dma_start(fb[:], src)
                    if b >= ACT_B:
                        nc.vector.tensor_reduce(
                            out=nl3[:, t0 : t0 + cnt, b],
                            in_=fb[:],
                            axis=mybir.AxisListType.X,
                            op=mybir.AluOpType.add,
                        )
                    else:
                        scratch = spool.tile(
                            [P, HW], f32, name="scr", tag="scr"
                        )
                        for j in range(cnt):
                            nc.scalar.activation(
                                out=scratch[:],
                                in_=fb[:, j, :],
                                func=mybir.ActivationFunctionType.Copy,
                                accum_out=nl3[:, t0 + j, b : b + 1],
                            )

                # AllGather this group's pooled neck slice
                ag_in = drampool.tile([P, cnt * B_LOC], f32, name=f"ag_in{g}")
                ag_out = drampool.tile(
                    [N_CORES * P, cnt * B_LOC], f32, name=f"ag_out{g}",
                    addr_space="Shared",
                )
                nc.scalar.dma_start(
                    ag_in[:], neck_local[:, t0 * B_LOC : (t0 + cnt) * B_LOC]
                )
                nc.gpsimd.collective_compute(
                    "AllGather",
                    mybir.AluOpType.bypass,
                    ins=[ag_in[:].opt()],
                    outs=[ag_out[:].opt()],
                    replica_groups=rg,
                )
                # read back the whole gathered group in ONE SWDGE DMA
                # (f32 -> bf16 cast): dst[p, c, r, b] <- ag_out[(r p), (c b)]
                ag4 = ag_out[:].rearrange("(r p) (c b) -> p c r b", p=P, b=B_LOC)
                dst4 = lhsT_g[g][:].rearrange("p c (r b) -> p c r b", b=B_LOC)
                nc.gpsimd.dma_start(dst4, ag4)

                # matmuls for this group's K-tiles (accumulate into psum)
                for j in range(cnt):
                    t = t0 + j
                    for m in range(2):
                        for nci, (noff, nsz) in enumerate(N_CHUNKS):
                            nc.tensor.matmul(
                                psum_tiles[(m, nci)][:],
                                lhsT=lhsT_g[g][:, j, m * P : (m + 1) * P],
                                rhs=w_sb[:, t, noff : noff + nsz],
                                start=(t == 0),
                                stop=(t == N_CT - 1),
                            )

            # epilogue: PSUM -> SBUF -> DRAM
            for m in range(2):
                for nci, (noff, nsz) in enumerate(N_CHUNKS):
                    osb = opool.tile([P, nsz], f32, name="osb", tag="osb")
                    if (m + nci) % 2 == 0:
                        nc.vector.tensor_copy(out=osb[:], in_=psum_tiles[(m, nci)][:])
                    else:
                        nc.scalar.copy(out=osb[:], in_=psum_tiles[(m, nci)][:])
                    nc.sync.dma_start(
                        logits_out.ap()[m, :, noff : noff + nsz], osb[:]
                    )
            nc.sync.dma_start(neck_out.ap()[:, :], neck_local[:])

    nc.compile()
    _CACHE["nc"] = nc
    return nc


def _channel_of_tile():
    """c[p, t] = global channel held at partition p of K-tile t."""
    bases = _group_bases()
    cmap = np.zeros((P, N_CT), dtype=np.int64)
    for g, cnt in enumerate(GROUPS):
        t0 = bases[g]
        for j in range(cnt):
            cmap[:, t0 + j] = t0 * P + np.arange(P) * cnt + j
    return cmap


def _shard_inputs(features, weight):
    features = np.ascontiguousarray(features, dtype=np.float32).reshape(B, C, HW)
    weight = np.asarray(weight, dtype=np.float32)
    in_maps = []
    for i in range(N_CORES):
        f_i = np.ascontiguousarray(features[i * B_LOC : (i + 1) * B_LOC])
        w_i = np.ascontiguousarray(
            weight[i * CLS_LOC : (i + 1) * CLS_LOC, :].T
        ) / np.float32(HW)
        in_maps.append({"features": f_i, "weight_t": w_i})
    return in_maps


def _assemble(results):
    # logits: core i holds classes [i*1250, (i+1)*1250) for all 256 rows
    logits = np.concatenate(
        [results[i]["logits"].reshape(B, CLS_LOC) for i in range(N_CORES)], axis=1
    ).astype(np.float32, copy=False)
    # neck: core i holds pooled SUM for batch rows [i*32, (i+1)*32), layout
    # [p, t, b] with channel cmap[p, t]
    cmap = _channel_of_tile()  # [P, N_CT]
    neck_parts = []
    for i in range(N_CORES):
        arr = results[i]["neck_out"].reshape(P, N_CT, B_LOC)
        part = np.empty((B_LOC, C), dtype=np.float32)
        # part[b, cmap[p,t]] = arr[p, t, b]
        part[:, cmap.reshape(-1)] = arr.reshape(P * N_CT, B_LOC).T
        neck_parts.append(part)
    neck = np.concatenate(neck_parts, axis=0) / np.float32(HW)
    return logits, neck


def _softmax_tail(logits, targets):
    """Faithful replication of the reference's softmax/new_weight path."""
    t = logits.shape[0]
    mx = logits.max(axis=1, keepdims=True)
    e = np.exp(logits - mx)
    denom = e.sum(axis=1)
    tgt = np.asarray(targets).astype(np.int64).reshape(-1)
    right_prob = e[np.arange(t), tgt] / denom  # [t]
    mean_sl = right_prob.astype(np.float32)
    var_sl = np.zeros_like(mean_sl)
    with np.errstate(divide="ignore", invalid="ignore"):
        con = mean_sl / (var_sl * np.float32(1e4))
    ri = np.tanh(np.float32(1.2) * con).astype(np.float32)
    new_weight = (np.float32(t) * ri / ri.sum())[None, :].astype(np.float32)
    return new_weight


def kernel(features, targets, weight, _trace=False, _extra=None):
    from concourse.bass_utils import run_bass_kernel_spmd

    nc = _build()
    in_maps = _shard_inputs(features, weight)
    res = run_bass_kernel_spmd(
        nc, in_maps, core_ids=list(range(N_CORES)), trace=_trace
    )
    if _extra is not None:
        _extra["bass_results"] = res
    logits, neck = _assemble(res.results)
    new_weight = _softmax_tail(logits, targets)
    cls_outputs = logits
    pred_class_logits = logits * np.float32(1.0)
    return cls_outputs, pred_class_logits, neck, new_weight
